# revision 23
# baseline (speedup 1.0000x reference)
"""Trainium2 Bass kernel for Bengio03HighwayBiLm.

Model: L=2 layers x 2 directions of [width-4 conv over sequence (H=512 -> 512)
+ ReLU + 2 highway sublayers (512 -> 1024 split into nonlin/gate)].

Sharding: data-parallel over batch across 8 cores (4 batches/core), weights
replicated. Channels-on-partitions layout ([ch, pos]); host pre-transposes.

vs the fp16 baseline:
- Highway GATE matmuls run as pure-fp8 DoubleRow (2x contraction/cycle): the
  sigmoid's derivative (<=0.25) attenuates fp8 quantization noise 4x, keeping
  end-to-end rms rel err ~1.2e-2 (sim-verified) under the 2e-2 gate. Conv and
  nonlin stay fp16 (noise there is unattenuated).
- Software-pipelined across (layer, dir, batch) units: conv of unit u+1 is
  issued between the two highway sublayers of unit u, so the serial
  act->combine chain of sublayer 0 hides under ~14us of conv matmuls.
- Merged [128, 4, 512] activation tiles: 3-op DVE combine, single fused
  mult+clip DVE cast for the fp8 gate input.
- Layer-0 outputs stay in SBUF as padded layer-1 conv inputs (no DRAM
  round-trip).
- Next block's weights prefetched one block ahead; output DMA alternates
  queues.
"""

import sys

for _p in ("/opt/trn_rl_repo", "/root/.axon_site/_ro/trn_rl_repo"):
    if _p not in sys.path:
        sys.path.append(_p)

from contextlib import ExitStack

import numpy as np
import ml_dtypes

import concourse.bass as bass
import concourse.tile as tile
from concourse import bacc, bass_utils, mybir

F32 = mybir.dt.float32
F16 = mybir.dt.float16
F8 = mybir.dt.float8e4
AF = mybir.ActivationFunctionType
ALU = mybir.AluOpType
DR = mybir.MatmulPerfMode.DoubleRow
NP8 = ml_dtypes.float8_e4m3

B, S, H = 32, 512, 512
L, NHW, WIDTH = 2, 2, 3
NCORES = 8
BL = B // NCORES          # batches per core
SP = S + 2 * WIDTH        # padded sequence length
HC = H // 128             # channel chunks

S_A = 16.0                # fp8 storage scale for gate activations
S_GW = 512.0              # fp8 storage scale for gate weights

BLOCKS = [(0, 0), (0, 1), (1, 0), (1, 1)]

_CACHE = {}


def _build():
    if "nc" in _CACHE:
        return _CACHE["nc"]

    nc = bacc.Bacc("TRN2", target_bir_lowering=False, debug=False,
                   num_devices=NCORES)

    x_t = nc.dram_tensor("x_t", [BL, H, SP], F16, kind="ExternalInput").ap()
    convw = nc.dram_tensor("convw", [L, 2, 4, 128, HC, 512], F16,
                           kind="ExternalInput").ap()
    hww = nc.dram_tensor("hww", [L, 2, NHW, 4, 128, HC, 128], F16,
                         kind="ExternalInput").ap()
    wg8 = nc.dram_tensor("wg8", [L, 2, NHW, 128, HC, 512], F8,
                         kind="ExternalInput").ap()
    convb = nc.dram_tensor("convb", [L, 2, 128, 4], F32,
                           kind="ExternalInput").ap()
    hwb = nc.dram_tensor("hwb", [L, 2, 128, NHW * 8], F32,
                         kind="ExternalInput").ap()
    padt = nc.dram_tensor("padt", [L, 2, 128, HC, 3], F16,
                          kind="ExternalInput").ap()
    out_t = nc.dram_tensor("out_t", [L, 2, BL, H, S], F16,
                           kind="ExternalOutput").ap()

    with tile.TileContext(nc) as tc, ExitStack() as ctx:
        sb = ctx.enter_context(tc.tile_pool(name="sb", bufs=2))
        ps = ctx.enter_context(tc.tile_pool(name="ps", bufs=8, space="PSUM"))

        # persistent padded layer-1 conv inputs, built in SBUF by layer 0
        xpad1 = [[sb.tile([128, HC, SP], F16, name=f"xpad1_{d}{b}",
                          tag="xpad1", bufs=2 * BL) for b in range(BL)]
                 for d in range(2)]

        weights = {}   # (li, d) -> dict of weight tiles
        units = [(li, d, b) for (li, d) in BLOCKS for b in range(BL)]
        ustate = {}    # u -> dict(xin=, hf=, xo0=)

        def load_weights(li, d, xin0=None):
            wc = [[None] * HC for _ in range(4)]
            for j in range(4):
                for hc in range(HC):
                    w = sb.tile([128, 512], F16, name=f"wc_{li}{d}{j}{hc}",
                                tag="wc", bufs=34)
                    if j == 0 and xin0 is not None:
                        # startup: interleave b0 input chunks with tap-0
                        # weights across both queues so the first matmul's
                        # operands land first; split the first chunks so
                        # transfers overlap
                        q = nc.sync if hc % 2 == 0 else nc.gpsimd
                        nsplit = 4 if hc == 0 else (2 if hc == 1 else 1)
                        step = SP // nsplit
                        for si in range(nsplit):
                            c0 = si * step
                            c1 = SP if si == nsplit - 1 else c0 + step
                            q.dma_start(
                                xin0[:, hc, c0:c1],
                                x_t[0, hc * 128:(hc + 1) * 128, c0:c1])
                        wsplit = 2 if hc < 2 else 1
                        wstep = 512 // wsplit
                        for si in range(wsplit):
                            q.dma_start(
                                w[:, si * wstep:(si + 1) * wstep],
                                convw[li, d, j, :, hc,
                                      si * wstep:(si + 1) * wstep])
                    else:
                        nc.sync.dma_start(w[:], convw[li, d, j, :, hc])
                    wc[j][hc] = w
            wh = []
            wgt = []
            for jh in range(NHW):
                row = []
                for gc in range(4):
                    t = sb.tile([128, HC, 128], F16,
                                name=f"wh_{li}{d}{jh}{gc}", tag="wh", bufs=17)
                    nc.gpsimd.dma_start(t[:], hww[li, d, jh, gc])
                    row.append(t)
                wh.append(row)
                g = sb.tile([128, HC, 512], F8, name=f"wgt_{li}{d}{jh}",
                            tag="wgt", bufs=5)
                nc.gpsimd.dma_start(g[:], wg8[li, d, jh])
                wgt.append(g)
            cb = sb.tile([128, 4], F32, name=f"cb_{li}{d}", tag="cb", bufs=4)
            nc.gpsimd.dma_start(cb[:], convb[li, d])
            hb = sb.tile([128, NHW * 8], F32, name=f"hb_{li}{d}", tag="hb",
                         bufs=4)
            nc.gpsimd.dma_start(hb[:], hwb[li, d])
            weights[(li, d)] = dict(wc=wc, wh=wh, wgt=wgt, cb=cb, hb=hb)

        def load_xin(li, d, b, split=False):
            if li == 1:
                return xpad1[d][b]
            t = sb.tile([128, HC, SP], F16, name=f"xin_{li}{d}{b}",
                        tag="xin", bufs=6)
            for hc in range(HC):
                q = nc.gpsimd if (split and hc % 2 == 1) else nc.sync
                q.dma_start(t[:, hc, :], x_t[b, hc * 128:(hc + 1) * 128, :])
            return t

        def emit_conv(u):
            li, d, b = u
            w = weights[(li, d)]
            off = 0 if d == 0 else WIDTH
            xin = ustate[u]["xin"]
            hf = sb.tile([128, 4, 512], F16, name=f"hf_{li}{d}{b}",
                         tag="hf", bufs=3)
            x8 = sb.tile([128, 4, 512], F8, name=f"x8_{li}{d}{b}0",
                         tag="x8", bufs=3)
            for oc in range(4):
                pt = ps.tile([128, 512], F32, name=f"cps_{li}{d}{b}{oc}",
                             tag="ps")
                k = 0
                for j in range(4):
                    for hc in range(HC):
                        nc.tensor.matmul(
                            pt[:],
                            w["wc"][j][hc][:, oc * 128:(oc + 1) * 128],
                            xin[:, hc, off + j:off + j + S],
                            start=(k == 0), stop=(k == 15))
                        k += 1
                nc.scalar.activation(hf[:, oc, :], pt[:], AF.Relu,
                                     bias=w["cb"][:, oc:oc + 1])
                # fp8 gate input for sublayer 0, off the critical path
                nc.vector.tensor_scalar(x8[:, oc, :], hf[:, oc, :], S_A,
                                        240.0, ALU.mult, ALU.min)
            ustate[u]["hf"] = hf
            ustate[u]["x8_0"] = x8

        def emit_hw(u, jh):
            li, d, b = u
            w = weights[(li, d)]
            xcur = ustate[u]["hf"] if jh == 0 else ustate[u]["xo0"]
            x8 = ustate[u][f"x8_{jh}"]
            # gates first: their sigmoid acts drain during the nonlin MMs
            gt = sb.tile([128, 4, 512], F16, name=f"gt_{li}{d}{b}{jh}",
                         tag="gt", bufs=3)
            for gc in range(4):
                pg = ps.tile([128, 512], F32,
                             name=f"gps_{li}{d}{b}{jh}{gc}", tag="ps")
                for p in range(2):
                    nc.tensor.matmul(
                        pg[:],
                        w["wgt"][jh][:, 2 * p:2 * p + 2,
                                     gc * 128:(gc + 1) * 128],
                        x8[:, 2 * p:2 * p + 2, :],
                        start=(p == 0), stop=(p == 1), perf_mode=DR)
                nc.scalar.activation(
                    gt[:, gc, :], pg[:], AF.Sigmoid,
                    bias=w["hb"][:, jh * 8 + 4 + gc:jh * 8 + 4 + gc + 1],
                    scale=1.0 / (S_A * S_GW))
            rt = sb.tile([128, 4, 512], F16, name=f"rt_{li}{d}{b}{jh}",
                         tag="rt", bufs=3)
            for gc in range(4):
                pt = ps.tile([128, 512], F32,
                             name=f"hps_{li}{d}{b}{jh}{gc}", tag="ps")
                for hc in range(HC):
                    nc.tensor.matmul(
                        pt[:],
                        w["wh"][jh][gc][:, hc, :],
                        xcur[:, hc, :],
                        start=(hc == 0), stop=(hc == HC - 1))
                nc.scalar.activation(
                    rt[:, gc, :], pt[:], AF.Relu,
                    bias=w["hb"][:, jh * 8 + gc:jh * 8 + gc + 1])

            # combine: x' = r + g*(x - r), per-hc ops; also cast the next
            # sublayer's fp8 gate input as each chunk completes
            if jh == NHW - 1 and li == 0:
                xo_t = xpad1[d][b]
                oc0 = WIDTH
            else:
                xo_t = sb.tile([128, 4, 512], F16,
                               name=f"xo_{li}{d}{b}{jh}", tag="xo", bufs=3)
                oc0 = 0
            if jh == 0:
                x8n = sb.tile([128, 4, 512], F8, name=f"x8_{li}{d}{b}1",
                              tag="x8", bufs=3)
                ustate[u]["x8_1"] = x8n
            for hc in range(HC):
                xoh = xo_t[:, hc, oc0:oc0 + S]
                nc.vector.tensor_sub(xoh, xcur[:, hc, :], rt[:, hc, :])
                nc.vector.tensor_mul(xoh, gt[:, hc, :], xoh)
                nc.vector.tensor_add(xoh, xoh, rt[:, hc, :])
                if jh == 0:
                    nc.vector.tensor_scalar(x8n[:, hc, :], xoh, S_A, 240.0,
                                            ALU.mult, ALU.min)

            if jh == 0:
                ustate[u]["xo0"] = xo_t
            else:
                o0 = WIDTH if li == 0 else 0
                last = (li == 1 and d == 1 and b == BL - 1)
                if last:
                    # tail: split into half-rows over three queues
                    qs = [nc.sync, nc.gpsimd, nc.scalar]
                    hm = S // 2
                    k = 0
                    for hc in range(HC):
                        for c0, c1 in ((0, hm), (hm, S)):
                            qs[k % 3].dma_start(
                                out_t[li, d, b,
                                      hc * 128:(hc + 1) * 128, c0:c1],
                                xo_t[:, hc, o0 + c0:o0 + c1])
                            k += 1
                else:
                    q = nc.sync if b % 2 == 0 else nc.gpsimd
                    for hc in range(HC):
                        q.dma_start(
                            out_t[li, d, b, hc * 128:(hc + 1) * 128, :],
                            xo_t[:, hc, o0:o0 + S])

        # ------- pipelined emission -------
        xin0 = sb.tile([128, HC, SP], F16, name="xin_000", tag="xin", bufs=6)
        load_weights(0, 0, xin0=xin0)
        ustate[(0, 0, 0)] = {"xin": xin0}
        for b in range(1, BL):
            ustate[(0, 0, b)] = {"xin": load_xin(0, 0, b)}
        # pad columns for layer-1 inputs (off critical path)
        padf = sb.tile([128, HC, 3], F16, name="padf", tag="padf", bufs=2)
        nc.gpsimd.dma_start(padf[:], padt[1, 0])
        padb = sb.tile([128, HC, 3], F16, name="padb", tag="padb", bufs=2)
        nc.gpsimd.dma_start(padb[:], padt[1, 1])
        for dd in range(2):
            for b in range(BL):
                nc.gpsimd.dma_start(xpad1[dd][b][:, :, 0:WIDTH], padf[:])
                nc.gpsimd.dma_start(xpad1[dd][b][:, :, WIDTH + S:SP],
                                    padb[:])

        for ui, u in enumerate(units):
            li, d, b = u
            if b == 1 and (li, d) != BLOCKS[-1]:
                # prefetch next block's weights + inputs one block ahead
                nli, nd = BLOCKS[BLOCKS.index((li, d)) + 1]
                load_weights(nli, nd)
                for nb in range(BL):
                    nu = (nli, nd, nb)
                    if nu not in ustate:
                        ustate[nu] = {"xin": load_xin(nli, nd, nb)}
            if u not in ustate:
                ustate[u] = {"xin": load_xin(li, d, b)}
            emit_conv(u)
            if ui >= 1:
                emit_hw(units[ui - 1], 1)
            emit_hw(u, 0)
        emit_hw(units[-1], 1)

    nc.compile()
    _CACHE["nc"] = nc
    return nc


def _prep_shared(fwd_pad, bwd_pad, fwd_w, fwd_b, bwd_w, bwd_b,
                 fwd_hw_w, fwd_hw_b, bwd_hw_w, bwd_hw_b):
    f32 = np.float32
    convw = np.empty((L, 2, 4, 128, HC, 512), np.float16)
    convb = np.empty((L, 2, 128, 4), f32)
    hww = np.empty((L, 2, NHW, 4, 128, HC, 128), np.float16)
    wg8 = np.empty((L, 2, NHW, 128, HC, 512), NP8)
    hwb = np.empty((L, 2, 128, NHW * 8), f32)
    padt = np.empty((L, 2, 128, HC, 3), np.float16)
    for li in range(L):
        for dd, (w, bia, hw_w, hw_b) in enumerate(
                [(fwd_w, fwd_b, fwd_hw_w, fwd_hw_b),
                 (bwd_w, bwd_b, bwd_hw_w, bwd_hw_b)]):
            # w[li]: [512o, 2048=(j,hc,p)] -> [j, p, hc, o]
            convw[li, dd] = w[li].reshape(512, 4, HC, 128).transpose(1, 3, 2, 0)
            convb[li, dd] = bia[li].reshape(4, 128).T
            for jh in range(NHW):
                # nonlin half: [512=(gc,gi), 512=(hc,p)] -> [gc, p, hc, gi]
                hww[li, dd, jh] = hw_w[li, jh][:512].reshape(
                    4, 128, HC, 128).transpose(0, 3, 2, 1)
                # gate half, fp8: [512g, 512=(hc,p)] -> [p, hc, g]
                wg = hw_w[li, jh][512:].reshape(512, HC, 128) \
                                       .transpose(2, 1, 0)
                wg8[li, dd, jh] = np.clip(
                    wg * S_GW, -240.0, 240.0).astype(NP8)
                hwb[li, dd][:, jh * 8:(jh + 1) * 8] = \
                    hw_b[li, jh].reshape(8, 128).T
        padt[li, 0] = fwd_pad[li].T.reshape(HC, 128, 3).transpose(1, 0, 2)
        padt[li, 1] = bwd_pad[li].T.reshape(HC, 128, 3).transpose(1, 0, 2)
    return dict(convw=convw, convb=convb, hww=hww, wg8=wg8, hwb=hwb,
                padt=padt)


def kernel(inputs, fwd_pad, bwd_pad, fwd_w, fwd_b, bwd_w, bwd_b,
           fwd_hw_w, fwd_hw_b, bwd_hw_w, bwd_hw_b, _trace=False):
    nc = _build()
    shared = _prep_shared(
        np.asarray(fwd_pad), np.asarray(bwd_pad),
        np.asarray(fwd_w), np.asarray(fwd_b),
        np.asarray(bwd_w), np.asarray(bwd_b),
        np.asarray(fwd_hw_w), np.asarray(fwd_hw_b),
        np.asarray(bwd_hw_w), np.asarray(bwd_hw_b))
    x = np.asarray(inputs, dtype=np.float32)

    in_maps = []
    for c in range(NCORES):
        xs = x[c * BL:(c + 1) * BL].transpose(0, 2, 1)  # [BL, H, S]
        xc = np.empty((BL, H, SP), np.float16)
        xc[:, :, WIDTH:WIDTH + S] = xs
        xc[:, :, 0:WIDTH] = np.asarray(fwd_pad)[0].T[None]
        xc[:, :, WIDTH + S:SP] = np.asarray(bwd_pad)[0].T[None]
        in_maps.append({"x_t": xc, **shared})

    res = bass_utils.run_bass_kernel_spmd(
        nc, in_maps, core_ids=list(range(NCORES)), trace=_trace)

    out = np.empty((L, B, S, 2 * H), np.float32)
    for c in range(NCORES):
        o = res.results[c]["out_t"].astype(np.float32)  # [L, 2, BL, H, S]
        for li in range(L):
            out[li, c * BL:(c + 1) * BL, :, :H] = o[li, 0].transpose(0, 2, 1)
            out[li, c * BL:(c + 1) * BL, :, H:] = o[li, 1].transpose(0, 2, 1)
    if _trace:
        kernel.last_exec_time_ns = res.exec_time_ns
        kernel.last_trace = (res.instructions_and_trace[1]
                             if res.instructions_and_trace else None)
    return out


# revision 25
# speedup vs baseline: 1.2144x; 1.2144x over previous
"""Trainium2 Bass kernel for Bengio03HighwayBiLm.

Model: L=2 layers x 2 directions of [width-4 conv over sequence (H=512 -> 512)
+ ReLU + 2 highway sublayers (512 -> 1024 split into nonlin/gate)].

Sharding: data-parallel over batch across 8 cores (4 batches/core), weights
replicated. Channels-on-partitions layout ([ch, pos]); host pre-transposes.

vs the fp16 baseline:
- Highway GATE matmuls run as pure-fp8 DoubleRow (2x contraction/cycle): the
  sigmoid's derivative (<=0.25) attenuates fp8 quantization noise 4x, keeping
  end-to-end rms rel err ~1.2e-2 (sim-verified) under the 2e-2 gate. Conv and
  nonlin stay fp16 (noise there is unattenuated).
- Software-pipelined across (layer, dir, batch) units: conv of unit u+1 is
  issued between the two highway sublayers of unit u, so the serial
  act->combine chain of sublayer 0 hides under ~14us of conv matmuls.
- Merged [128, 4, 512] activation tiles: 3-op DVE combine, single fused
  mult+clip DVE cast for the fp8 gate input.
- Layer-0 outputs stay in SBUF as padded layer-1 conv inputs (no DRAM
  round-trip).
- Next block's weights prefetched one block ahead; output DMA alternates
  queues.
"""

import sys

for _p in ("/opt/trn_rl_repo", "/root/.axon_site/_ro/trn_rl_repo"):
    if _p not in sys.path:
        sys.path.append(_p)

from contextlib import ExitStack

import numpy as np
import ml_dtypes

import concourse.bass as bass
import concourse.tile as tile
from concourse import bacc, bass_utils, mybir

F32 = mybir.dt.float32
F16 = mybir.dt.float16
F8 = mybir.dt.float8e4
AF = mybir.ActivationFunctionType
ALU = mybir.AluOpType
DR = mybir.MatmulPerfMode.DoubleRow
NP8 = ml_dtypes.float8_e4m3

B, S, H = 32, 512, 512
L, NHW, WIDTH = 2, 2, 3
NCORES = 8
BL = B // NCORES          # batches per core
SP = S + 2 * WIDTH        # padded sequence length
HC = H // 128             # channel chunks

S_A = 16.0                # fp8 storage scale for gate activations
S_GW = 512.0              # fp8 storage scale for gate weights

BLOCKS = [(0, 0), (0, 1), (1, 0), (1, 1)]

_CACHE = {}


def _build():
    if "nc" in _CACHE:
        return _CACHE["nc"]

    nc = bacc.Bacc("TRN2", target_bir_lowering=False, debug=False,
                   num_devices=NCORES)

    x_t = nc.dram_tensor("x_t", [BL, H, SP], F16, kind="ExternalInput").ap()
    convw = nc.dram_tensor("convw", [L, 2, 4, 128, HC, 512], F16,
                           kind="ExternalInput").ap()
    hww = nc.dram_tensor("hww", [L, 2, NHW, 4, 128, HC, 128], F16,
                         kind="ExternalInput").ap()
    wg8 = nc.dram_tensor("wg8", [L, 2, NHW, 128, HC, 512], F8,
                         kind="ExternalInput").ap()
    convb = nc.dram_tensor("convb", [L, 2, 128, 4], F32,
                           kind="ExternalInput").ap()
    hwb = nc.dram_tensor("hwb", [L, 2, 128, NHW * 8], F32,
                         kind="ExternalInput").ap()
    padt = nc.dram_tensor("padt", [L, 2, 128, HC, 3], F16,
                          kind="ExternalInput").ap()
    out_t = nc.dram_tensor("out_t", [L, 2, BL, H, S], F16,
                           kind="ExternalOutput").ap()

    with tile.TileContext(nc) as tc, ExitStack() as ctx:
        sb = ctx.enter_context(tc.tile_pool(name="sb", bufs=2))
        ps = ctx.enter_context(tc.tile_pool(name="ps", bufs=8, space="PSUM"))

        # persistent padded layer-1 conv inputs, built in SBUF by layer 0
        xpad1 = [[sb.tile([128, HC, SP], F16, name=f"xpad1_{d}{b}",
                          tag="xpad1", bufs=2 * BL) for b in range(BL)]
                 for d in range(2)]

        weights = {}   # (li, d) -> dict of weight tiles
        units = [(li, d, b) for (li, d) in BLOCKS for b in range(BL)]
        ustate = {}    # u -> dict(xin=, hf=, xo0=)

        def load_weights(li, d, xin0=None):
            wc = [[None] * HC for _ in range(4)]
            for j in range(4):
                for hc in range(HC):
                    w = sb.tile([128, 512], F16, name=f"wc_{li}{d}{j}{hc}",
                                tag="wc", bufs=34)
                    if j == 0 and xin0 is not None:
                        # startup: interleave b0 input chunks with tap-0
                        # weights across both queues so the first matmul's
                        # operands land first
                        q = nc.sync if hc % 2 == 0 else nc.gpsimd
                        q.dma_start(xin0[:, hc, :],
                                    x_t[0, hc * 128:(hc + 1) * 128, :])
                        q.dma_start(w[:], convw[li, d, j, :, hc])
                    else:
                        nc.sync.dma_start(w[:], convw[li, d, j, :, hc])
                    wc[j][hc] = w
            wh = []
            wgt = []
            for jh in range(NHW):
                row = []
                for gc in range(4):
                    t = sb.tile([128, HC, 128], F16,
                                name=f"wh_{li}{d}{jh}{gc}", tag="wh", bufs=17)
                    nc.gpsimd.dma_start(t[:], hww[li, d, jh, gc])
                    row.append(t)
                wh.append(row)
                g = sb.tile([128, HC, 512], F8, name=f"wgt_{li}{d}{jh}",
                            tag="wgt", bufs=5)
                nc.gpsimd.dma_start(g[:], wg8[li, d, jh])
                wgt.append(g)
            cb = sb.tile([128, 4], F32, name=f"cb_{li}{d}", tag="cb", bufs=4)
            nc.gpsimd.dma_start(cb[:], convb[li, d])
            hb = sb.tile([128, NHW * 8], F32, name=f"hb_{li}{d}", tag="hb",
                         bufs=4)
            nc.gpsimd.dma_start(hb[:], hwb[li, d])
            weights[(li, d)] = dict(wc=wc, wh=wh, wgt=wgt, cb=cb, hb=hb)

        def load_xin(li, d, b, split=False):
            if li == 1:
                return xpad1[d][b]
            t = sb.tile([128, HC, SP], F16, name=f"xin_{li}{d}{b}",
                        tag="xin", bufs=6)
            for hc in range(HC):
                q = nc.gpsimd if (split and hc % 2 == 1) else nc.sync
                q.dma_start(t[:, hc, :], x_t[b, hc * 128:(hc + 1) * 128, :])
            return t

        def emit_conv(u):
            li, d, b = u
            w = weights[(li, d)]
            off = 0 if d == 0 else WIDTH
            xin = ustate[u]["xin"]
            hf = sb.tile([128, 4, 512], F16, name=f"hf_{li}{d}{b}",
                         tag="hf", bufs=3)
            x8 = sb.tile([128, 4, 512], F8, name=f"x8_{li}{d}{b}0",
                         tag="x8", bufs=3)
            for oc in range(4):
                pt = ps.tile([128, 512], F32, name=f"cps_{li}{d}{b}{oc}",
                             tag="ps")
                k = 0
                for j in range(4):
                    for hc in range(HC):
                        nc.tensor.matmul(
                            pt[:],
                            w["wc"][j][hc][:, oc * 128:(oc + 1) * 128],
                            xin[:, hc, off + j:off + j + S],
                            start=(k == 0), stop=(k == 15))
                        k += 1
                nc.scalar.activation(hf[:, oc, :], pt[:], AF.Relu,
                                     bias=w["cb"][:, oc:oc + 1])
                # fp8 gate input for sublayer 0, off the critical path
                nc.vector.tensor_scalar(x8[:, oc, :], hf[:, oc, :], S_A,
                                        240.0, ALU.mult, ALU.min)
            ustate[u]["hf"] = hf
            ustate[u]["x8_0"] = x8

        def emit_hw(u, jh):
            li, d, b = u
            w = weights[(li, d)]
            xcur = ustate[u]["hf"] if jh == 0 else ustate[u]["xo0"]
            x8 = ustate[u][f"x8_{jh}"]
            # gates first: their sigmoid acts drain during the nonlin MMs
            gt = sb.tile([128, 4, 512], F16, name=f"gt_{li}{d}{b}{jh}",
                         tag="gt", bufs=3)
            for gc in range(4):
                pg = ps.tile([128, 512], F32,
                             name=f"gps_{li}{d}{b}{jh}{gc}", tag="ps")
                for p in range(2):
                    nc.tensor.matmul(
                        pg[:],
                        w["wgt"][jh][:, 2 * p:2 * p + 2,
                                     gc * 128:(gc + 1) * 128],
                        x8[:, 2 * p:2 * p + 2, :],
                        start=(p == 0), stop=(p == 1), perf_mode=DR)
                nc.scalar.activation(
                    gt[:, gc, :], pg[:], AF.Sigmoid,
                    bias=w["hb"][:, jh * 8 + 4 + gc:jh * 8 + 4 + gc + 1],
                    scale=1.0 / (S_A * S_GW))
            rt = sb.tile([128, 4, 512], F16, name=f"rt_{li}{d}{b}{jh}",
                         tag="rt", bufs=3)
            for gc in range(4):
                pt = ps.tile([128, 512], F32,
                             name=f"hps_{li}{d}{b}{jh}{gc}", tag="ps")
                for hc in range(HC):
                    nc.tensor.matmul(
                        pt[:],
                        w["wh"][jh][gc][:, hc, :],
                        xcur[:, hc, :],
                        start=(hc == 0), stop=(hc == HC - 1))
                nc.scalar.activation(
                    rt[:, gc, :], pt[:], AF.Relu,
                    bias=w["hb"][:, jh * 8 + gc:jh * 8 + gc + 1])

            # combine: x' = r + g*(x - r), per-hc ops; also cast the next
            # sublayer's fp8 gate input as each chunk completes
            if jh == NHW - 1 and li == 0:
                xo_t = xpad1[d][b]
                oc0 = WIDTH
            else:
                xo_t = sb.tile([128, 4, 512], F16,
                               name=f"xo_{li}{d}{b}{jh}", tag="xo", bufs=3)
                oc0 = 0
            if jh == 0:
                x8n = sb.tile([128, 4, 512], F8, name=f"x8_{li}{d}{b}1",
                              tag="x8", bufs=3)
                ustate[u]["x8_1"] = x8n
            for hc in range(HC):
                xoh = xo_t[:, hc, oc0:oc0 + S]
                nc.vector.tensor_sub(xoh, xcur[:, hc, :], rt[:, hc, :])
                nc.vector.tensor_mul(xoh, gt[:, hc, :], xoh)
                nc.vector.tensor_add(xoh, xoh, rt[:, hc, :])
                if jh == 0:
                    nc.vector.tensor_scalar(x8n[:, hc, :], xoh, S_A, 240.0,
                                            ALU.mult, ALU.min)

            if jh == 0:
                ustate[u]["xo0"] = xo_t
            else:
                o0 = WIDTH if li == 0 else 0
                last = (li == 1 and d == 1 and b == BL - 1)
                qs = ([nc.sync, nc.gpsimd, nc.scalar, nc.sync] if last
                      else [nc.sync if b % 2 == 0 else nc.gpsimd] * 4)
                for hc in range(HC):
                    qs[hc % len(qs)].dma_start(
                        out_t[li, d, b, hc * 128:(hc + 1) * 128, :],
                        xo_t[:, hc, o0:o0 + S])

        # ------- pipelined emission -------
        xin0 = sb.tile([128, HC, SP], F16, name="xin_000", tag="xin", bufs=6)
        load_weights(0, 0, xin0=xin0)
        ustate[(0, 0, 0)] = {"xin": xin0}
        for b in range(1, BL):
            ustate[(0, 0, b)] = {"xin": load_xin(0, 0, b)}
        # pad columns for layer-1 inputs (off critical path)
        padf = sb.tile([128, HC, 3], F16, name="padf", tag="padf", bufs=2)
        nc.gpsimd.dma_start(padf[:], padt[1, 0])
        padb = sb.tile([128, HC, 3], F16, name="padb", tag="padb", bufs=2)
        nc.gpsimd.dma_start(padb[:], padt[1, 1])
        for dd in range(2):
            for b in range(BL):
                nc.gpsimd.dma_start(xpad1[dd][b][:, :, 0:WIDTH], padf[:])
                nc.gpsimd.dma_start(xpad1[dd][b][:, :, WIDTH + S:SP],
                                    padb[:])

        for ui, u in enumerate(units):
            li, d, b = u
            if b == 1 and (li, d) != BLOCKS[-1]:
                # prefetch next block's weights + inputs one block ahead
                nli, nd = BLOCKS[BLOCKS.index((li, d)) + 1]
                load_weights(nli, nd)
                for nb in range(BL):
                    nu = (nli, nd, nb)
                    if nu not in ustate:
                        ustate[nu] = {"xin": load_xin(nli, nd, nb)}
            if u not in ustate:
                ustate[u] = {"xin": load_xin(li, d, b)}
            emit_conv(u)
            if ui >= 1:
                emit_hw(units[ui - 1], 1)
            emit_hw(u, 0)
        emit_hw(units[-1], 1)

    nc.compile()
    _CACHE["nc"] = nc
    return nc


def _prep_shared(fwd_pad, bwd_pad, fwd_w, fwd_b, bwd_w, bwd_b,
                 fwd_hw_w, fwd_hw_b, bwd_hw_w, bwd_hw_b):
    f32 = np.float32
    convw = np.empty((L, 2, 4, 128, HC, 512), np.float16)
    convb = np.empty((L, 2, 128, 4), f32)
    hww = np.empty((L, 2, NHW, 4, 128, HC, 128), np.float16)
    wg8 = np.empty((L, 2, NHW, 128, HC, 512), NP8)
    hwb = np.empty((L, 2, 128, NHW * 8), f32)
    padt = np.empty((L, 2, 128, HC, 3), np.float16)
    for li in range(L):
        for dd, (w, bia, hw_w, hw_b) in enumerate(
                [(fwd_w, fwd_b, fwd_hw_w, fwd_hw_b),
                 (bwd_w, bwd_b, bwd_hw_w, bwd_hw_b)]):
            # w[li]: [512o, 2048=(j,hc,p)] -> [j, p, hc, o]
            convw[li, dd] = w[li].reshape(512, 4, HC, 128).transpose(1, 3, 2, 0)
            convb[li, dd] = bia[li].reshape(4, 128).T
            for jh in range(NHW):
                # nonlin half: [512=(gc,gi), 512=(hc,p)] -> [gc, p, hc, gi]
                hww[li, dd, jh] = hw_w[li, jh][:512].reshape(
                    4, 128, HC, 128).transpose(0, 3, 2, 1)
                # gate half, fp8: [512g, 512=(hc,p)] -> [p, hc, g]
                wg = hw_w[li, jh][512:].reshape(512, HC, 128) \
                                       .transpose(2, 1, 0)
                wg8[li, dd, jh] = np.clip(
                    wg * S_GW, -240.0, 240.0).astype(NP8)
                hwb[li, dd][:, jh * 8:(jh + 1) * 8] = \
                    hw_b[li, jh].reshape(8, 128).T
        padt[li, 0] = fwd_pad[li].T.reshape(HC, 128, 3).transpose(1, 0, 2)
        padt[li, 1] = bwd_pad[li].T.reshape(HC, 128, 3).transpose(1, 0, 2)
    return dict(convw=convw, convb=convb, hww=hww, wg8=wg8, hwb=hwb,
                padt=padt)


def kernel(inputs, fwd_pad, bwd_pad, fwd_w, fwd_b, bwd_w, bwd_b,
           fwd_hw_w, fwd_hw_b, bwd_hw_w, bwd_hw_b, _trace=False):
    nc = _build()
    shared = _prep_shared(
        np.asarray(fwd_pad), np.asarray(bwd_pad),
        np.asarray(fwd_w), np.asarray(fwd_b),
        np.asarray(bwd_w), np.asarray(bwd_b),
        np.asarray(fwd_hw_w), np.asarray(fwd_hw_b),
        np.asarray(bwd_hw_w), np.asarray(bwd_hw_b))
    x = np.asarray(inputs, dtype=np.float32)

    in_maps = []
    for c in range(NCORES):
        xs = x[c * BL:(c + 1) * BL].transpose(0, 2, 1)  # [BL, H, S]
        xc = np.empty((BL, H, SP), np.float16)
        xc[:, :, WIDTH:WIDTH + S] = xs
        xc[:, :, 0:WIDTH] = np.asarray(fwd_pad)[0].T[None]
        xc[:, :, WIDTH + S:SP] = np.asarray(bwd_pad)[0].T[None]
        in_maps.append({"x_t": xc, **shared})

    res = bass_utils.run_bass_kernel_spmd(
        nc, in_maps, core_ids=list(range(NCORES)), trace=_trace)

    out = np.empty((L, B, S, 2 * H), np.float32)
    for c in range(NCORES):
        o = res.results[c]["out_t"].astype(np.float32)  # [L, 2, BL, H, S]
        for li in range(L):
            out[li, c * BL:(c + 1) * BL, :, :H] = o[li, 0].transpose(0, 2, 1)
            out[li, c * BL:(c + 1) * BL, :, H:] = o[li, 1].transpose(0, 2, 1)
    if _trace:
        kernel.last_exec_time_ns = res.exec_time_ns
        kernel.last_trace = (res.instructions_and_trace[1]
                             if res.instructions_and_trace else None)
    return out
